# revision 19
# baseline (speedup 1.0000x reference)
"""DGE block kernel for 8 Trainium2 NeuronCores.

Sharding: data-parallel over (b, n-half): core k handles batch b=k//2,
n-half h=k%2. Each core receives x[b] with its own n-half rolled to the
front, computes the full adjacency A[b] (mean over all N via its own
half resident in SBUF plus a streamed read of the partner half), and
produces H_out[b, :, half, :].

v2 design notes:
- the core's own half of x (16.8MB) is DMA'd once into a persistent
  SBUF tile `xall` [128=(blk,c), t=128, d=256] (n = blk*128 + t) with
  32KB-contiguous packets; H_out tiles are written back into consumed
  xall slots so the output leaves in 4 chunky DMAs.
- FFN matmuls run in bf16 (fp32r costs 2 cycles/row on HW); the
  propagate matmul and Hc accumulation stay fp32r; the residual path
  stays fp32, so only the FFN contribution carries bf16 noise.
- LN stats scalar chains (sqrt/reciprocal/-m*inv) are batched 4 tiles
  at a time; apply/cast work alternates between ACT and DVE.
"""

import sys

import numpy as np

B, C, N, D, DK, DFF = 4, 64, 512, 256, 64, 1024
NH = N // 2  # per-core n-half
TQ = NH // 2  # 128 t-tiles of 128 tokens, n = blk*128 + t
NGROUP = TQ // 4  # 32 groups of 512 tokens
EPS_LN, EPS_DEG = 1e-5, 1e-6
ATT_SCALE = 8.0  # sqrt(DK) * TAU
NCORES = 8

_CACHE = {}


def _ensure_path():
    try:
        import concourse  # noqa: F401
    except ImportError:
        sys.path.insert(0, "/opt/trn_rl_repo")


def _build_program(flags):
    """Build the Bass/Tile program. flags = (g1_trivial, b2_zero, g2_trivial)."""
    from contextlib import ExitStack

    import concourse.bacc as bacc
    import concourse.bass as bass
    import concourse.tile as tile
    from concourse import mybir
    from concourse.masks import make_identity

    g1_trivial, b2_zero, g2_trivial = flags

    fp32 = mybir.dt.float32
    fp32r = mybir.dt.float32r
    bf16 = mybir.dt.bfloat16
    F = mybir.ActivationFunctionType
    OP = mybir.AluOpType

    nc = bacc.Bacc("TRN2", target_bir_lowering=False, debug=False,
                   enable_asserts=False, num_devices=NCORES)

    xs = nc.dram_tensor("xs", [C, N, D], fp32r, kind="ExternalInput").ap()
    a_st = nc.dram_tensor("a_st", [C, C], fp32, kind="ExternalInput").ap()
    wqk = nc.dram_tensor("wqk", [D, 2 * DK], fp32r, kind="ExternalInput").ap()
    w1 = nc.dram_tensor("w1", [D, DFF], bf16, kind="ExternalInput").ap()
    b1 = nc.dram_tensor("b1", [DFF], fp32, kind="ExternalInput").ap()
    w2 = nc.dram_tensor("w2", [DFF, D], bf16, kind="ExternalInput").ap()
    b2 = None if b2_zero else nc.dram_tensor("b2", [D], fp32, kind="ExternalInput").ap()
    g2be2 = None
    if not g2_trivial:
        g2be2 = nc.dram_tensor("g2be2", [2, D], fp32, kind="ExternalInput").ap()
    g1be1 = None
    if not g1_trivial:
        g1be1 = nc.dram_tensor("g1be1", [2, D], fp32, kind="ExternalInput").ap()
    yh = nc.dram_tensor("yh", [C, NH, D], fp32, kind="ExternalOutput").ap()
    a_out = nc.dram_tensor("a_out", [C, C], fp32, kind="ExternalOutput").ap()

    def bcast_load(dst, src_ap, offset=0):
        nc.sync.dma_start(
            out=dst,
            in_=bass.AP(tensor=src_ap.tensor, offset=src_ap.offset + offset,
                        ap=[[0, 128], [1, D]]))

    with tile.TileContext(nc) as tc:
        with ExitStack() as ctx:
            consts = ctx.enter_context(tc.tile_pool(name="consts", bufs=1))
            xallp = ctx.enter_context(tc.tile_pool(name="xallp", bufs=1))

            id128 = consts.tile([128, 128], fp32)
            make_identity(nc, id128)
            eye64 = id128[0:64, 0:64]

            # off-diagonal mask (1 everywhere, 0 on diag)
            mask64 = consts.tile([64, 64], fp32)
            nc.gpsimd.memset(mask64, 1.0)
            nc.gpsimd.affine_select(
                out=mask64, in_=mask64,
                compare_op=OP.not_equal, fill=0.0,
                base=0, pattern=[[-1, 64]], channel_multiplier=1,
            )

            # sliding indicator for Hc: ind[:, 63] = 1/N, else 0.
            # lhsT for channel c = ind[:, 63-c : 127-c]  (col c == 1/N)
            ind_f = consts.tile([128, 127], fp32)
            nc.vector.memset(ind_f, 0.0)
            nc.vector.memset(ind_f[:, 63:64], 1.0 / N)
            ind = consts.tile([128, 127], fp32r)
            nc.vector.tensor_copy(ind, ind_f)

            # stacked-eye folder: E2[p, c] = (1/N) * (p % 64 == c)
            e2f = consts.tile([128, 64], fp32)
            nc.gpsimd.memset(e2f, 0.0)
            make_identity(nc, e2f[0:64, :], nomemset=True)
            make_identity(nc, e2f[64:128, :], nomemset=True)
            nc.vector.tensor_scalar_mul(e2f, e2f, 1.0 / N)
            e2 = consts.tile([128, 64], fp32r)
            nc.vector.tensor_copy(e2, e2f)

            eps1 = consts.tile([128, 1], fp32)
            nc.vector.memset(eps1, EPS_LN)

            w1sb = consts.tile([128, 2, DFF], bf16)
            nc.sync.dma_start(out=w1sb, in_=w1.rearrange("(k p) f -> p k f", k=2))
            w2sb = consts.tile([128, 8, D], bf16)
            nc.sync.dma_start(out=w2sb, in_=w2.rearrange("(k p) f -> p k f", k=8))
            wqksb = consts.tile([128, 2, 2 * DK], fp32r)
            nc.sync.dma_start(out=wqksb, in_=wqk.rearrange("(k p) f -> p k f", k=2))
            b1sb = consts.tile([128, 8], fp32)
            nc.sync.dma_start(out=b1sb, in_=b1.rearrange("(k p) -> p k", k=8))
            assb = consts.tile([64, 64], fp32)
            nc.sync.dma_start(out=assb, in_=a_st)
            bd = consts.tile([128, 128], fp32r)

            b2bc = None
            if b2 is not None:
                b2bc = consts.tile([128, D], fp32)
                bcast_load(b2bc, b2)
            g2bc = be2bc = None
            if g2be2 is not None:
                g2bc = consts.tile([128, D], fp32)
                be2bc = consts.tile([128, D], fp32)
                bcast_load(g2bc, g2be2)
                bcast_load(be2bc, g2be2, D)
            g1bc = be1bc = None
            if g1be1 is not None:
                g1bc = consts.tile([128, D], fp32)
                be1bc = consts.tile([128, D], fp32)
                bcast_load(g1bc, g1be1)
                bcast_load(be1bc, g1be1, D)

            # persistent own-half x: [p=(blk,c), t, d], n = blk*128 + t
            xall = xallp.tile([128, TQ, D], fp32r)
            for k in range(4):
                nc.sync.dma_start(
                    out=xall[:, 32 * k:32 * (k + 1), :],
                    in_=bass.AP(
                        tensor=xs.tensor, offset=xs.offset + 32 * k * D,
                        ap=[[TQ * D, 2], [N * D, C], [D, 32], [1, D]]))

            # ---------------- adjacency phase ----------------
            with ExitStack() as adj_ctx:
                adjp = adj_ctx.enter_context(tc.tile_pool(name="adjp", bufs=2))
                prtp = adj_ctx.enter_context(tc.tile_pool(name="prtp", bufs=4))
                adjps = adj_ctx.enter_context(
                    tc.tile_pool(name="adjps", bufs=1, space="PSUM"))

                # Hc accumulation into one [64, 512] bank:
                #   cols 0:256   += own-half partial + partner nl=0
                #   cols 256:512 += partner nl=1
                hc_ps = adjps.tile([64, 2, D], fp32)

                # partner half: per channel, one contiguous 256KB read,
                # reduced over partitions by the sliding-indicator matmul
                first = True
                for c in range(C):
                    prt = prtp.tile([128, 2 * D], fp32r, tag="prt")
                    nc.sync.dma_start(
                        out=prt,
                        in_=bass.AP(tensor=xs.tensor,
                                    offset=xs.offset + (c * N + NH) * D,
                                    ap=[[2 * D, 128], [1, 2 * D]]))
                    nc.tensor.matmul(hc_ps, ind[:, 63 - c:127 - c], prt,
                                     start=first, stop=False)
                    first = False

                # own half: reduce over t on DVE, fold (blk,c) -> c on PE
                ownp = adjp.tile([128, D], fp32r)
                with nc.allow_low_precision(reason="fp32r round of fp32 sum"):
                    nc.vector.tensor_reduce(
                        ownp, xall.rearrange("p t d -> p d t"),
                        axis=mybir.AxisListType.X, op=OP.add)
                nc.tensor.matmul(hc_ps[:, 0, :], e2, ownp,
                                 start=False, stop=True)

                hc_sb = adjp.tile([64, D], fp32)
                nc.vector.tensor_copy(hc_sb, hc_ps[:, 0, :])
                nc.vector.tensor_add(hc_sb, hc_sb, hc_ps[:, 1, :])

                # HcT [d(2x128), c]
                hcT = adjp.tile([128, 2, 64], fp32r)
                for j in range(2):
                    tp = adjps.tile([128, 64], fp32, tag="tp")
                    nc.tensor.transpose(tp, hc_sb[:, j * 128:(j + 1) * 128], eye64)
                    nc.vector.tensor_copy(hcT[:, j, :], tp)

                # QT, KT [a=64, c=64]
                qt_ps = adjps.tile([64, 64], fp32)
                kt_ps = adjps.tile([64, 64], fp32)
                for ch in range(2):
                    nc.tensor.matmul(qt_ps, wqksb[:, ch, 0:64], hcT[:, ch, :],
                                     start=(ch == 0), stop=(ch == 1))
                for ch in range(2):
                    nc.tensor.matmul(kt_ps, wqksb[:, ch, 64:128], hcT[:, ch, :],
                                     start=(ch == 0), stop=(ch == 1))
                qt_sb = adjp.tile([64, 64], fp32r)
                nc.vector.tensor_copy(qt_sb, qt_ps)
                kt_sb = adjp.tile([64, 64], fp32r)
                nc.vector.tensor_copy(kt_sb, kt_ps)

                # S = Q @ K^T ; dA = tanh(S/8)
                s_ps = adjps.tile([64, 64], fp32)
                nc.tensor.matmul(s_ps, qt_sb, kt_sb, start=True, stop=True)
                da_sb = adjp.tile([64, 64], fp32)
                nc.scalar.activation(da_sb, s_ps, F.Tanh, scale=1.0 / ATT_SCALE)

                # symmetrize: afull = a_static + 0.5*(dA + dA^T); zero diag
                dat_ps = adjps.tile([64, 64], fp32)
                nc.tensor.transpose(dat_ps, da_sb, eye64)
                t1 = adjp.tile([64, 64], fp32)
                nc.vector.tensor_add(t1, da_sb, dat_ps)
                afull = adjp.tile([64, 64], fp32)
                nc.vector.scalar_tensor_tensor(afull, t1, 0.5, assb,
                                               op0=OP.mult, op1=OP.add)
                nc.vector.tensor_mul(afull, afull, mask64)

                # degree normalize: An = diag(dis) A diag(dis)
                deg = adjp.tile([64, 1], fp32)
                nc.vector.tensor_reduce(deg, afull, axis=mybir.AxisListType.X,
                                        op=OP.add)
                nc.vector.tensor_scalar_max(deg, deg, EPS_DEG)
                sq = adjp.tile([64, 1], fp32)
                nc.scalar.activation(sq, deg, F.Sqrt)
                dis = adjp.tile([64, 1], fp32)
                nc.vector.reciprocal(dis, sq)
                m1 = adjp.tile([64, 64], fp32)
                nc.vector.tensor_scalar_mul(m1, afull, dis)
                m1t_ps = adjps.tile([64, 64], fp32)
                nc.tensor.transpose(m1t_ps, m1, eye64)
                an_sb = adjp.tile([64, 64], fp32)
                nc.vector.tensor_scalar_mul(an_sb, m1t_ps, dis)
                nc.sync.dma_start(out=a_out, in_=an_sb)

                # blockdiag(A+I, A+I) for the propagate matmul
                at_sb = adjp.tile([64, 64], fp32)
                nc.vector.tensor_add(at_sb, an_sb, eye64)
                bd_f = adjp.tile([128, 128], fp32)
                nc.vector.memset(bd_f, 0.0)
                nc.sync.dma_start(out=bd_f[0:64, 0:64], in_=at_sb)
                nc.sync.dma_start(out=bd_f[64:128, 64:128], in_=at_sb)
                nc.vector.tensor_copy(bd, bd_f)

            # ---------------- main loop ----------------
            hnp = ctx.enter_context(tc.tile_pool(name="hnp", bufs=3))
            hntp = ctx.enter_context(tc.tile_pool(name="hntp", bufs=3))
            gtp = ctx.enter_context(tc.tile_pool(name="gtp", bufs=2))
            r2p = ctx.enter_context(tc.tile_pool(name="r2p", bufs=5))
            statp = ctx.enter_context(tc.tile_pool(name="statp", bufs=4))
            psap = ctx.enter_context(tc.tile_pool(name="psap", bufs=2, space="PSUM"))
            z1p = ctx.enter_context(tc.tile_pool(name="z1p", bufs=2, space="PSUM"))
            o2p = ctx.enter_context(tc.tile_pool(name="o2p", bufs=2, space="PSUM"))

            def batched_inv(mvb, tag):
                """sqrt/recip/-m*inv for 4 tiles at once -> (inv4, nb4)."""
                sd4 = statp.tile([128, 4], fp32, tag=f"sd{tag}")
                nc.scalar.activation(sd4, mvb[:, :, 1], F.Sqrt, bias=eps1)
                inv4 = statp.tile([128, 4], fp32, tag=f"inv{tag}")
                nc.vector.reciprocal(inv4, sd4)
                nb4 = statp.tile([128, 4], fp32, tag=f"nb{tag}")
                nc.vector.tensor_mul(nb4, mvb[:, :, 0], inv4)
                nc.vector.tensor_scalar_mul(nb4, nb4, -1.0)
                return inv4, nb4

            for g in range(NGROUP):
                t0 = 4 * g

                # propagate (A+I) @ x -> PSUM, then LN1 stats
                mvb = statp.tile([128, 4, 2], fp32, tag="mvb")
                psas = []
                for u2 in range(2):
                    psa = psap.tile([128, 2, D], fp32, tag="psa")
                    psas.append(psa)
                    for u in range(2):
                        i = u2 * 2 + u
                        nc.tensor.matmul(psa[:, u, :], bd, xall[:, t0 + i, :],
                                         start=True, stop=True)
                    for u in range(2):
                        i = u2 * 2 + u
                        st = statp.tile([128, 6], fp32, tag="st")
                        nc.vector.bn_stats(st, psa[:, u, :])
                        nc.vector.bn_aggr(mvb[:, i, :], st)

                inv4, nb4 = batched_inv(mvb, "a")
                hn = hnp.tile([128, 4, D], fp32, tag="hn")
                for i in range(4):
                    nc.scalar.activation(
                        hn[:, i, :], psas[i // 2][:, i % 2, :], F.Identity,
                        bias=nb4[:, i:i + 1], scale=inv4[:, i:i + 1])

                # transpose hn -> hnT [d(2x128), tok=512] in bf16
                hnT = hntp.tile([128, 2, 512], bf16, tag="hnT")
                for i in range(4):
                    tp = psap.tile([128, 2, 256], fp32, tag="psa")
                    for dch in range(2):
                        nc.tensor.transpose(
                            tp[:, dch, 0:128],
                            hn[:, i, dch * 128:(dch + 1) * 128], id128)
                    dst = hnT[:, :, i * 128:(i + 1) * 128]
                    if i % 2 == 0:
                        nc.vector.tensor_copy(dst, tp[:, :, 0:128])
                    else:
                        nc.scalar.activation(dst, tp[:, :, 0:128], F.Identity)

                # FFN1 + gelu (bf16)
                gT = gtp.tile([128, 8, 512], bf16, tag="gT")
                for ff in range(8):
                    z1 = z1p.tile([128, 512], fp32, tag="z1")
                    nc.tensor.matmul(z1, w1sb[:, 0, ff * 128:(ff + 1) * 128],
                                     hnT[:, 0, :], start=True, stop=False)
                    nc.tensor.matmul(z1, w1sb[:, 1, ff * 128:(ff + 1) * 128],
                                     hnT[:, 1, :], start=False, stop=True)
                    nc.scalar.activation(gT[:, ff, :], z1, F.Gelu,
                                         bias=b1sb[:, ff:ff + 1])

                # FFN2 (bf16): per PSUM bank sequential accumulation groups
                o2 = o2p.tile([128, 4, D], fp32, tag="o2")
                for i in range(4):
                    for ff in range(8):
                        nc.tensor.matmul(
                            o2[:, i, :],
                            gT[:, ff, i * 128:(i + 1) * 128],
                            w2sb[:, ff, :],
                            start=(ff == 0), stop=(ff == 7))

                # residual2 + LN2, H_out written back into xall slots
                mv2b = statp.tile([128, 4, 2], fp32, tag="mv2b")
                r2s = []
                for i in range(4):
                    r2 = r2p.tile([128, D], fp32, tag="r2")
                    r2s.append(r2)
                    if g1_trivial:
                        nc.vector.scalar_tensor_tensor(
                            r2, o2[:, i, :], 1.0, hn[:, i, :],
                            op0=OP.mult, op1=OP.add)
                    else:
                        h1t = r2p.tile([128, D], fp32, tag="h1t")
                        nc.vector.tensor_mul(h1t, hn[:, i, :], g1bc)
                        nc.vector.tensor_add(h1t, h1t, be1bc)
                        nc.vector.tensor_add(r2, o2[:, i, :], h1t)
                    if b2bc is not None:
                        nc.vector.tensor_add(r2, r2, b2bc)
                    st2 = statp.tile([128, 6], fp32, tag="st2")
                    nc.vector.bn_stats(st2, r2)
                    nc.vector.bn_aggr(mv2b[:, i, :], st2)

                inv24, nb24 = batched_inv(mv2b, "b")
                for i in range(4):
                    dst = xall[:, t0 + i, :]
                    if g2_trivial and i % 2 == 0:
                        nc.vector.tensor_scalar(
                            dst, r2s[i], mv2b[:, i, 0:1], inv24[:, i:i + 1],
                            op0=OP.subtract, op1=OP.mult)
                    else:
                        nc.scalar.activation(
                            dst, r2s[i], F.Identity,
                            bias=nb24[:, i:i + 1], scale=inv24[:, i:i + 1])
                        if not g2_trivial:
                            nc.vector.tensor_mul(dst, dst, g2bc)
                            nc.vector.tensor_add(dst, dst, be2bc)

                if g % 8 == 7:
                    k = g // 8
                    nc.sync.dma_start(
                        out=bass.AP(
                            tensor=yh.tensor, offset=yh.offset + 32 * k * D,
                            ap=[[TQ * D, 2], [NH * D, C], [D, 32], [1, D]]),
                        in_=xall[:, 32 * k:32 * (k + 1), :].bitcast(fp32))

    nc.compile()
    return nc


def _prep_inputs(x, A_static, Wq, Wk, W1, b1, W2, b2, g1, be1, g2, be2):
    import ml_dtypes
    bf16 = ml_dtypes.bfloat16

    x = np.ascontiguousarray(np.asarray(x, dtype=np.float32))
    A_static = np.asarray(A_static, dtype=np.float32)
    Wq = np.asarray(Wq, dtype=np.float32)
    Wk = np.asarray(Wk, dtype=np.float32)
    W1 = np.asarray(W1, dtype=np.float32)
    b1 = np.asarray(b1, dtype=np.float32)
    W2 = np.asarray(W2, dtype=np.float32)
    b2 = np.asarray(b2, dtype=np.float32)
    g1 = np.asarray(g1, dtype=np.float32)
    be1 = np.asarray(be1, dtype=np.float32)
    g2 = np.asarray(g2, dtype=np.float32)
    be2 = np.asarray(be2, dtype=np.float32)

    g1_trivial = bool(np.all(g1 == 1.0) and np.all(be1 == 0.0))
    g2_trivial = bool(np.all(g2 == 1.0) and np.all(be2 == 0.0))
    b2_zero = bool(np.all(b2 == 0.0))
    flags = (g1_trivial, b2_zero, g2_trivial)

    # fold g1/be1 into the first FFN matmul (exact)
    if g1_trivial:
        W1f = W1
        b1f = b1
    else:
        W1f = (g1[:, None] * W1).astype(np.float32)
        b1f = (b1 + be1 @ W1).astype(np.float32)
    W1bf = np.ascontiguousarray(W1f.astype(bf16))
    W2bf = np.ascontiguousarray(W2.astype(bf16))
    Wqk = np.ascontiguousarray(np.concatenate([Wq, Wk], axis=1))

    in_maps = []
    for k in range(NCORES):
        b, h = divmod(k, 2)
        if h == 0:
            xsh = x[b]
        else:
            xsh = np.ascontiguousarray(
                np.concatenate([x[b, :, NH:, :], x[b, :, :NH, :]], axis=1))
        m = dict(xs=xsh, a_st=A_static, wqk=Wqk, w1=W1bf, b1=b1f, w2=W2bf)
        if not b2_zero:
            m["b2"] = b2
        if not g2_trivial:
            m["g2be2"] = np.stack([g2, be2])
        if not g1_trivial:
            m["g1be1"] = np.stack([g1, be1])
        in_maps.append(m)
    return flags, in_maps, x


def _get_executor(flags):
    """Build (once) and return fn(in_maps) -> list[dict] running on 8 cores."""
    if flags in _CACHE:
        return _CACHE[flags]

    _ensure_path()
    import jax
    from jax.sharding import Mesh, PartitionSpec
    try:
        from jax.experimental.shard_map import shard_map
    except ImportError:
        from jax.shard_map import shard_map
    from concourse import bass2jax as b2j
    from concourse import mybir

    nc = _build_program(flags)
    b2j.install_neuronx_cc_hook()

    partition_name = (nc.partition_id_tensor.name
                      if nc.partition_id_tensor else None)

    in_names, out_names, out_avals, zero_shapes = [], [], [], []
    for alloc in nc.m.functions[0].allocations:
        if not isinstance(alloc, mybir.MemoryLocationSet):
            continue
        name = alloc.memorylocations[0].name
        if alloc.kind == "ExternalInput":
            if name != partition_name:
                in_names.append(name)
        elif alloc.kind == "ExternalOutput":
            shape = tuple(alloc.tensor_shape)
            dtype = mybir.dt.np(alloc.dtype)
            out_names.append(name)
            out_avals.append(jax.core.ShapedArray(shape, dtype))
            zero_shapes.append((shape, dtype))

    n_params = len(in_names)
    n_outs = len(out_names)
    all_names = list(in_names) + list(out_names)
    if partition_name is not None:
        all_names.append(partition_name)
    donate = tuple(range(n_params, n_params + n_outs))

    def _body(*args):
        operands = list(args)
        if partition_name is not None:
            operands.append(b2j.partition_id_tensor())
        outs = b2j._bass_exec_p.bind(
            *operands,
            out_avals=tuple(out_avals),
            in_names=tuple(all_names),
            out_names=tuple(out_names),
            lowering_input_output_aliases=(),
            sim_require_finite=True,
            sim_require_nnan=True,
            nc=nc,
        )
        return tuple(outs)

    devices = jax.devices()[:NCORES]
    mesh = Mesh(np.asarray(devices), ("core",))
    in_specs = (PartitionSpec("core"),) * (n_params + n_outs)
    out_specs = (PartitionSpec("core"),) * n_outs
    sharded = jax.jit(
        shard_map(_body, mesh=mesh, in_specs=in_specs, out_specs=out_specs,
                  check_rep=False),
        donate_argnums=donate, keep_unused=True)

    def run(in_maps):
        assert len(in_maps) == NCORES
        concat_in = [
            np.concatenate([np.asarray(m[name]) for m in in_maps], axis=0)
            for name in in_names
        ]
        concat_zeros = [
            np.zeros((NCORES * sh[0], *sh[1:]), dt) for sh, dt in zero_shapes
        ]
        out_arrs = sharded(*concat_in, *concat_zeros)
        out_np = [np.asarray(a) for a in out_arrs]
        return [
            {name: out_np[i].reshape(NCORES, *zero_shapes[i][0])[c]
             for i, name in enumerate(out_names)}
            for c in range(NCORES)
        ]

    _CACHE[flags] = run
    return run


def kernel(x, A_static, Wq, Wk, W1, b1, W2, b2, g1, be1, g2, be2):
    flags, in_maps, x = _prep_inputs(x, A_static, Wq, Wk, W1, b1, W2, b2,
                                     g1, be1, g2, be2)
    run = _get_executor(flags)
    results = run(in_maps)

    H_out = np.empty((B, C, N, D), dtype=np.float32)
    A = np.empty((B, C, C), dtype=np.float32)
    for k in range(NCORES):
        b, h = divmod(k, 2)
        H_out[b, :, h * NH:(h + 1) * NH, :] = results[k]["yh"]
        if h == 0:
            A[b] = results[k]["a_out"]
    return H_out, x, A
